# revision 11
# baseline (speedup 1.0000x reference)
"""MultiOutSizeLinear (MoE-style routed linear) for Trainium2, 8 NeuronCores.

Each token selects one of 4 experts by its ``out_feat_size`` value
(128/256/512/1024). Expert k is a dense [out_k, 1024] linear + bias whose
output lands in the first out_k columns of the 1024-wide output row; the
reference leaves bias[k, out_k:] in the remaining columns (zero for the
shipped setup_inputs, which pre-zeroes the bias tail).

Strategy
  host:   route tokens to experts; balance each expert's tokens evenly
          across the 8 cores (capacities are shared so one SPMD program
          serves all cores); gather + transpose each core's tokens into
          x^T [1024, TPAD] laid out as expert segments [e3 | e2 | e1 | e0].
  device: keep W^T [1024, 1920] (all experts, concatenated out-columns) and
          a 128-row broadcast bias resident in SBUF. Stream 512-token
          chunks of x^T over the ACT HWDGE ring. All tensors that feed the
          PE are declared float32r (raw fp32 bits; the PE's full-rate fp32
          mode, ~1.3e-4 relative error vs fp32). Experts 1-3 run
          token-stationary: psum[128 tok, out_k] += xT_tile.T @ wT_tile,
          8 accumulating K-tiles per <=512-wide column chunk. Expert 0
          (out=128, too narrow for full-rate f32r) runs weight-stationary:
          psum[128 out, 512 tok] = out0^T chunks. Bias is added on VectorE
          during PSUM eviction (expert 0's bias is added on the host).
          Compact per-expert outputs go back over the SP HWDGE ring.
  host:   scatter rows back through the routing permutation.
"""

import sys
import numpy as np

sys.path.insert(0, "/opt/trn_rl_repo")

OUT_SIZES = (128, 256, 512, 1024)
N_EXPERTS = len(OUT_SIZES)
IN_FEAT = 1024
N_CORES = 8
K_TILES = IN_FEAT // 128
CHUNK = 512  # tokens per x^T DMA
WOFF = tuple(int(np.cumsum((0,) + OUT_SIZES)[k]) for k in range(N_EXPERTS))
W_COLS = sum(OUT_SIZES)

_nc_cache: dict = {}


def _build(caps, repeat=1, loop=None, xbufs=6, obufs=4):
    """Compile the SPMD program for shared per-expert capacities ``caps``.

    caps[0] % 512 == 0, caps[1]+caps[2]+caps[3] % 512 == 0, each % 128 == 0.
    ``repeat``/``loop`` re-run the compute body (same I/O) for timing.
    """
    import concourse.bacc as bacc
    import concourse.mybir as mybir
    import concourse.tile as tile

    f32 = mybir.dt.float32
    bf16 = mybir.dt.bfloat16
    tpad = sum(caps)
    assert tpad % CHUNK == 0 and caps[0] % 512 == 0
    assert (caps[1] + caps[2] + caps[3]) % 512 == 0

    nc = bacc.Bacc(None, target_bir_lowering=False, debug=False)
    # chunk-blocked x^T: block c holds tokens [c*CHUNK, (c+1)*CHUNK) as a
    # contiguous [IN_FEAT, CHUNK] slab -> each chunk DMA is one fully
    # sequential 2 MB HBM read (strided reads measured ~1.8x slower)
    xt = nc.dram_tensor("xt", [tpad // CHUNK, IN_FEAT, CHUNK], bf16,
                        kind="ExternalInput")
    wt = nc.dram_tensor("wt", [IN_FEAT, W_COLS], bf16, kind="ExternalInput")
    bb = nc.dram_tensor("bb", [128, W_COLS], f32, kind="ExternalInput")
    outs = {}
    for k in (1, 2, 3):
        if caps[k]:
            outs[k] = nc.dram_tensor(f"out{k}", [caps[k], OUT_SIZES[k]], bf16,
                                     kind="ExternalOutput")
    if caps[0]:
        outs[0] = nc.dram_tensor("out0t", [128, caps[0]], bf16,
                                 kind="ExternalOutput")

    seg_order = [k for k in (3, 2, 1, 0) if caps[k] > 0]
    seg_start = {}
    t0 = 0
    for k in seg_order:
        seg_start[k] = t0
        t0 += caps[k]

    def expert_of(tok):
        for k in seg_order:
            if tok < seg_start[k] + caps[k]:
                return k
        raise AssertionError


    with tile.TileContext(nc) as tc:
        with (
            tc.tile_pool(name="const", bufs=1) as const,
            tc.tile_pool(name="xp", bufs=xbufs) as xp,
            tc.tile_pool(name="op", bufs=obufs) as op,
            tc.tile_pool(name="ps", bufs=3, space="PSUM") as psp,
            tc.tile_pool(name="ps0", bufs=2, space="PSUM") as psp0,
        ):
            wt_sb = const.tile([128, K_TILES, W_COLS], bf16)
            nc.sync.dma_start(wt_sb[:], wt.rearrange("(kk p) n -> p kk n", p=128))
            bb_sb = const.tile([128, W_COLS], f32)
            nc.sync.dma_start(bb_sb[:], bb[:])

            def body():
                for c0 in range(0, tpad, CHUNK):
                    x_sb = xp.tile([128, K_TILES, CHUNK], bf16, tag="x")
                    nc.scalar.dma_start(
                        x_sb[:],
                        xt[c0 // CHUNK].rearrange("(kk p) t -> p kk t", p=128))
                    g0 = 0
                    while g0 < CHUNK:
                        tok = c0 + g0
                        k = expert_of(tok)
                        if k == 0:
                            # weight-stationary: psum = out0^T [128 out, 512 tok]
                            ps = psp0.tile([128, 512], f32, tag="ps0")
                            for kk in range(K_TILES):
                                nc.tensor.matmul(
                                    ps[:],
                                    wt_sb[:, kk, WOFF[0]:WOFF[0] + 128],
                                    x_sb[:, kk, g0:g0 + 512],
                                    start=(kk == 0), stop=(kk == K_TILES - 1))
                            o_sb = op.tile([128, 512], bf16, tag="o0")
                            nc.vector.tensor_copy(o_sb[:], ps[:])
                            row = tok - seg_start[0]
                            nc.sync.dma_start(outs[0][:, row:row + 512], o_sb[:])
                            g0 += 512
                            continue
                        ok = OUT_SIZES[k]
                        ps = psp.tile([128, 1024], f32, tag="ps")
                        for j0 in range(0, ok, 512):
                            jn = min(512, ok - j0)
                            for kk in range(K_TILES):
                                nc.tensor.matmul(
                                    ps[:, j0:j0 + jn],
                                    x_sb[:, kk, g0:g0 + 128],
                                    wt_sb[:, kk, WOFF[k] + j0:WOFF[k] + j0 + jn],
                                    start=(kk == 0), stop=(kk == K_TILES - 1))
                        o_sb = op.tile([128, 1024], bf16, tag="o")
                        nc.vector.tensor_add(o_sb[:, :ok], ps[:, :ok],
                                             bb_sb[:, WOFF[k]:WOFF[k] + ok])
                        row = tok - seg_start[k]
                        nc.sync.dma_start(outs[k][row:row + 128, :], o_sb[:, :ok])
                        g0 += 128

            if loop:
                with tc.For_i(0, loop, 1):
                    body()
            else:
                for _ in range(repeat):
                    body()
    nc.compile()
    return nc


def _get_nc(caps, repeat=1, loop=None):
    key = (tuple(caps), repeat, loop)
    if key not in _nc_cache:
        _nc_cache[key] = _build(caps, repeat=repeat, loop=loop)
    return _nc_cache[key]


def _route(out_feat_size):
    """Map out_feat_size values -> expert index (-1 = matches no expert)."""
    ofs = np.asarray(out_feat_size).astype(np.int64).reshape(-1)
    branch = np.full(ofs.shape, -1, dtype=np.int64)
    for k, s in enumerate(OUT_SIZES):
        branch[ofs == s] = k
    return branch


def _plan(branch):
    """Balanced routing plan: per-expert global index lists split evenly
    across cores, shared capacities, and segment layout [3,2,1,0]."""
    idx_all = {k: np.nonzero(branch == k)[0] for k in range(N_EXPERTS)}
    per_core = [int(-(-len(idx_all[k]) // N_CORES)) for k in range(N_EXPERTS)]
    caps = [int(-(-per_core[k] // 128) * 128) for k in range(N_EXPERTS)]
    # alignment: caps0 % 512, (caps1+2+3) % 512
    if caps[0] % 512:
        caps[0] += 512 - caps[0] % 512
    rem = (caps[1] + caps[2] + caps[3]) % 512
    if rem:
        for k in (1, 2, 3):  # pad the cheapest non-empty of e1..e3
            if caps[k]:
                caps[k] += 512 - rem
                break
        else:
            caps[0] += (512 - rem) if caps[0] else 0
    return idx_all, tuple(caps)


def kernel(x, weight, bias, out_feat_size):
    import ml_dtypes
    from concourse.bass_utils import run_bass_kernel_spmd

    bf16 = np.dtype(ml_dtypes.bfloat16)
    x = np.asarray(x, dtype=np.float32)
    weight = np.asarray(weight, dtype=np.float32)
    bias = np.asarray(bias, dtype=np.float32)
    B, T, D = x.shape
    assert D == IN_FEAT
    n_tok = B * T

    branch = _route(out_feat_size)
    idx_all, caps = _plan(branch)
    if sum(caps) == 0:
        return np.zeros((B, T, IN_FEAT), dtype=np.float32)

    # host-side weight/bias layout
    wt = np.empty((IN_FEAT, W_COLS), dtype=np.float32)
    bb = np.empty((W_COLS,), dtype=np.float32)
    for k, ok in enumerate(OUT_SIZES):
        wt[:, WOFF[k]:WOFF[k] + ok] = weight[k, :ok, :].T
        bb[WOFF[k]:WOFF[k] + ok] = bias[k, :ok]
    wt = wt.astype(bf16)
    bb128 = np.ascontiguousarray(np.broadcast_to(bb, (128, W_COLS)))

    x2 = x.reshape(n_tok, IN_FEAT).astype(bf16)
    tpad = sum(caps)
    seg_off = {}
    t0 = 0
    for k in (3, 2, 1, 0):
        if caps[k]:
            seg_off[k] = t0
            t0 += caps[k]

    in_maps = []
    core_slices = []  # per core: {expert: global idx array}
    for c in range(N_CORES):
        perm = np.zeros(tpad, dtype=np.int64)
        slices = {}
        for k, off in seg_off.items():
            idx = idx_all[k]
            m = int(-(-len(idx) // N_CORES))
            part = idx[c * m:(c + 1) * m]
            slices[k] = part
            if len(part):
                perm[off:off + len(part)] = part
                perm[off + len(part):off + caps[k]] = part[0]
        xtb = np.empty((tpad // CHUNK, IN_FEAT, CHUNK), dtype=bf16)
        for ci in range(tpad // CHUNK):
            np.copyto(xtb[ci], x2[perm[ci * CHUNK:(ci + 1) * CHUNK]].T)
        in_maps.append({"xt": xtb, "wt": wt, "bb": bb128})
        core_slices.append(slices)

    global _LAST_CAPS, _LAST_IN_MAPS
    _LAST_CAPS, _LAST_IN_MAPS = caps, in_maps

    nc = _get_nc(caps)
    res = run_bass_kernel_spmd(nc, in_maps, list(range(N_CORES))).results

    out = np.zeros((n_tok, IN_FEAT), dtype=np.float32)
    for c in range(N_CORES):
        for k, part in core_slices[c].items():
            n = len(part)
            if n == 0:
                continue
            ok = OUT_SIZES[k]
            if k == 0:
                out[part, :ok] = (res[c]["out0t"][:, :n].T.astype(np.float32)
                                  + bias[0, :ok])
            else:
                out[part, :ok] = res[c][f"out{k}"][:n].astype(np.float32)
            if ok < IN_FEAT:
                # reference semantics: bias tail beyond out_k (zero for the
                # shipped inputs, which pre-zero the bias)
                out[part, ok:] = bias[k, ok:]
    return out.reshape(B, T, IN_FEAT)



# revision 13
# speedup vs baseline: 1.0443x; 1.0443x over previous
"""MultiOutSizeLinear (MoE-style routed linear) for Trainium2, 8 NeuronCores.

Each token selects one of 4 experts by its ``out_feat_size`` value
(128/256/512/1024). Expert k is a dense [out_k, 1024] linear + bias whose
output lands in the first out_k columns of the 1024-wide output row; the
reference leaves bias[k, out_k:] in the remaining columns (zero for the
shipped setup_inputs, which pre-zeroes the bias tail).

Strategy
  host:   route tokens to experts; balance each expert's tokens evenly
          across the 8 cores (capacities are shared so one SPMD program
          serves all cores); gather + transpose each core's tokens into
          x^T [1024, TPAD] laid out as expert segments [e3 | e2 | e1 | e0];
          convert x and W to bf16 (the PE's full-rate dtype; rel err ~3e-3,
          well inside the 2e-2 gate).
  device: keep W^T [1024, 1920] (all experts, concatenated out-columns) and
          a 128-row broadcast bias resident in SBUF. Stream 512-token
          chunks of x^T over the ACT HWDGE ring. The whole kernel is
          PE-bound (measured: DMA fully hides under the matmul stream), so
          the matmul stream is structured to keep two independent PSUM
          accumulation chains in flight at all times (alternating PSUM
          banks) - consecutive matmuls never serialize on one chain's
          fill/drain, which measurably lifts the PE clock-gate (HAM)
          throughput on this part. Layouts per expert:
            e3 (1024 out): per 128-token group, the two 512-col halves
                alternate; both chains share the same stationary x tile.
            e2/e1:         two 128-token groups run in parallel chains on
                           separate banks (single-chain fallback at ragged
                           segment edges).
            e0 (128 out):  weight-stationary out^T chains [128 out, 512
                           tok]; two consecutive 512-token chunks paired.
          Bias is added on VectorE during PSUM eviction (expert 0's bias is
          added on the host). Compact bf16 per-expert outputs go back over
          the SP HWDGE ring.
  host:   scatter rows back through the routing permutation, restore f32.
"""

import sys
import numpy as np

sys.path.insert(0, "/opt/trn_rl_repo")

OUT_SIZES = (128, 256, 512, 1024)
N_EXPERTS = len(OUT_SIZES)
IN_FEAT = 1024
N_CORES = 8
K_TILES = IN_FEAT // 128
CHUNK = 512  # tokens per x^T DMA
WOFF = tuple(int(np.cumsum((0,) + OUT_SIZES)[k]) for k in range(N_EXPERTS))
W_COLS = sum(OUT_SIZES)

_nc_cache: dict = {}


def _build(caps, repeat=1, loop=None, xbufs=6, obufs=4):
    """Compile the SPMD program for shared per-expert capacities ``caps``.

    caps[0] % 512 == 0, caps[1]+caps[2]+caps[3] % 512 == 0, each % 128 == 0.
    ``repeat``/``loop`` re-run the compute body (same I/O) for timing.
    """
    import concourse.bacc as bacc
    import concourse.mybir as mybir
    import concourse.tile as tile

    f32 = mybir.dt.float32
    bf16 = mybir.dt.bfloat16
    tpad = sum(caps)
    assert tpad % CHUNK == 0 and caps[0] % 512 == 0
    assert (caps[1] + caps[2] + caps[3]) % 512 == 0

    nc = bacc.Bacc(None, target_bir_lowering=False, debug=False)
    # chunk-blocked x^T: block c holds tokens [c*CHUNK, (c+1)*CHUNK) as a
    # contiguous [IN_FEAT, CHUNK] slab -> each chunk DMA is one fully
    # sequential 1 MB HBM read (strided reads measured ~1.8x slower)
    xt = nc.dram_tensor("xt", [tpad // CHUNK, IN_FEAT, CHUNK], bf16,
                        kind="ExternalInput")
    wt = nc.dram_tensor("wt", [IN_FEAT, W_COLS], bf16, kind="ExternalInput")
    bb = nc.dram_tensor("bb", [128, W_COLS], f32, kind="ExternalInput")
    outs = {}
    for k in (1, 2, 3):
        if caps[k]:
            outs[k] = nc.dram_tensor(f"out{k}", [caps[k], OUT_SIZES[k]], bf16,
                                     kind="ExternalOutput")
    if caps[0]:
        outs[0] = nc.dram_tensor("out0t", [128, caps[0]], bf16,
                                 kind="ExternalOutput")

    seg_order = [k for k in (3, 2, 1, 0) if caps[k] > 0]
    seg_start = {}
    t0 = 0
    for k in seg_order:
        seg_start[k] = t0
        t0 += caps[k]

    def expert_of(tok):
        for k in seg_order:
            if tok < seg_start[k] + caps[k]:
                return k
        raise AssertionError

    with tile.TileContext(nc) as tc:
        with (
            tc.tile_pool(name="const", bufs=1) as const,
            tc.tile_pool(name="xp", bufs=xbufs) as xp,
            tc.tile_pool(name="op", bufs=obufs) as op,
            tc.tile_pool(name="ps", bufs=4, space="PSUM") as psp,
        ):
            wt_sb = const.tile([128, K_TILES, W_COLS], bf16)
            nc.sync.dma_start(wt_sb[:], wt.rearrange("(kk p) n -> p kk n", p=128))
            bb_sb = const.tile([128, W_COLS], f32)
            nc.sync.dma_start(bb_sb[:], bb[:])

            def body():
                e0_prev = None  # (x_sb, row) of an unpaired e0 chunk
                for c0 in range(0, tpad, CHUNK):
                    x_sb = xp.tile([128, K_TILES, CHUNK], bf16, tag="x")
                    nc.scalar.dma_start(
                        x_sb[:],
                        xt[c0 // CHUNK].rearrange("(kk p) t -> p kk t", p=128))
                    g0 = 0
                    while g0 < CHUNK:
                        tok = c0 + g0
                        k = expert_of(tok)
                        if k == 0:
                            # weight-stationary out^T chains; pair chunks so
                            # two chains interleave on separate PSUM banks
                            if e0_prev is None:
                                e0_prev = (x_sb, tok - seg_start[0])
                                g0 += 512
                                continue
                            xa_sb, rowa = e0_prev
                            e0_prev = None
                            rowb = tok - seg_start[0]
                            ps = psp.tile([128, 1024], f32, tag="ps")
                            for kk in range(K_TILES):
                                nc.tensor.matmul(
                                    ps[:, 0:512],
                                    wt_sb[:, kk, WOFF[0]:WOFF[0] + 128],
                                    xa_sb[:, kk, 0:512],
                                    start=(kk == 0), stop=(kk == K_TILES - 1))
                                nc.tensor.matmul(
                                    ps[:, 512:1024],
                                    wt_sb[:, kk, WOFF[0]:WOFF[0] + 128],
                                    x_sb[:, kk, g0:g0 + 512],
                                    start=(kk == 0), stop=(kk == K_TILES - 1))
                            o_sb = op.tile([128, 1024], bf16, tag="o0")
                            nc.vector.tensor_copy(o_sb[:], ps[:])
                            assert rowb == rowa + 512
                            nc.sync.dma_start(outs[0][:, rowa:rowa + 1024],
                                              o_sb[:])
                            g0 += 512
                            continue
                        ok = OUT_SIZES[k]
                        ps = psp.tile([128, 1024], f32, tag="ps")
                        if ok == 1024:
                            # two 512-col chains of one group, shared lhsT
                            for kk in range(K_TILES):
                                nc.tensor.matmul(
                                    ps[:, 0:512],
                                    x_sb[:, kk, g0:g0 + 128],
                                    wt_sb[:, kk, WOFF[3]:WOFF[3] + 512],
                                    start=(kk == 0), stop=(kk == K_TILES - 1))
                                nc.tensor.matmul(
                                    ps[:, 512:1024],
                                    x_sb[:, kk, g0:g0 + 128],
                                    wt_sb[:, kk, WOFF[3] + 512:WOFF[3] + 1024],
                                    start=(kk == 0), stop=(kk == K_TILES - 1))
                            o_sb = op.tile([128, 1024], bf16, tag="o")
                            nc.vector.tensor_add(o_sb[:], ps[:],
                                                 bb_sb[:, WOFF[3]:WOFF[3] + 1024])
                            row = tok - seg_start[3]
                            nc.sync.dma_start(outs[3][row:row + 128, :],
                                              o_sb[:])
                            g0 += 128
                            continue
                        # e1/e2: pair two 128-token groups on separate banks;
                        # fall back to one chain at ragged segment edges
                        n_here = min(seg_start[k] + caps[k] - tok,
                                     CHUNK - g0) // 128
                        row = tok - seg_start[k]
                        if n_here >= 2:
                            ga, gb = g0, g0 + 128
                            for kk in range(K_TILES):
                                nc.tensor.matmul(
                                    ps[:, 0:ok],
                                    x_sb[:, kk, ga:ga + 128],
                                    wt_sb[:, kk, WOFF[k]:WOFF[k] + ok],
                                    start=(kk == 0), stop=(kk == K_TILES - 1))
                                nc.tensor.matmul(
                                    ps[:, 512:512 + ok],
                                    x_sb[:, kk, gb:gb + 128],
                                    wt_sb[:, kk, WOFF[k]:WOFF[k] + ok],
                                    start=(kk == 0), stop=(kk == K_TILES - 1))
                            o_sb = op.tile([128, 1024], bf16, tag="o")
                            nc.vector.tensor_add(o_sb[:, 0:ok], ps[:, 0:ok],
                                                 bb_sb[:, WOFF[k]:WOFF[k] + ok])
                            nc.vector.tensor_add(o_sb[:, 512:512 + ok],
                                                 ps[:, 512:512 + ok],
                                                 bb_sb[:, WOFF[k]:WOFF[k] + ok])
                            nc.sync.dma_start(outs[k][row:row + 128, :],
                                              o_sb[:, 0:ok])
                            nc.sync.dma_start(outs[k][row + 128:row + 256, :],
                                              o_sb[:, 512:512 + ok])
                            g0 += 256
                        else:
                            for kk in range(K_TILES):
                                nc.tensor.matmul(
                                    ps[:, 0:ok],
                                    x_sb[:, kk, g0:g0 + 128],
                                    wt_sb[:, kk, WOFF[k]:WOFF[k] + ok],
                                    start=(kk == 0), stop=(kk == K_TILES - 1))
                            o_sb = op.tile([128, 1024], bf16, tag="o")
                            nc.vector.tensor_add(o_sb[:, 0:ok], ps[:, 0:ok],
                                                 bb_sb[:, WOFF[k]:WOFF[k] + ok])
                            nc.sync.dma_start(outs[k][row:row + 128, :],
                                              o_sb[:, 0:ok])
                            g0 += 128
                if e0_prev is not None:
                    # odd number of e0 chunks: single-chain tail
                    xa_sb, rowa = e0_prev
                    ps = psp.tile([128, 1024], f32, tag="ps")
                    for kk in range(K_TILES):
                        nc.tensor.matmul(
                            ps[:, 0:512],
                            wt_sb[:, kk, WOFF[0]:WOFF[0] + 128],
                            xa_sb[:, kk, 0:512],
                            start=(kk == 0), stop=(kk == K_TILES - 1))
                    o_sb = op.tile([128, 1024], bf16, tag="o0")
                    nc.vector.tensor_copy(o_sb[:, 0:512], ps[:, 0:512])
                    nc.sync.dma_start(outs[0][:, rowa:rowa + 512],
                                      o_sb[:, 0:512])

            if loop:
                with tc.For_i(0, loop, 1):
                    for _ in range(repeat):
                        body()
            else:
                for _ in range(repeat):
                    body()
    nc.compile()
    return nc


def _get_nc(caps, repeat=1, loop=None):
    key = (tuple(caps), repeat, loop)
    if key not in _nc_cache:
        _nc_cache[key] = _build(caps, repeat=repeat, loop=loop)
    return _nc_cache[key]


def _route(out_feat_size):
    """Map out_feat_size values -> expert index (-1 = matches no expert)."""
    ofs = np.asarray(out_feat_size).astype(np.int64).reshape(-1)
    branch = np.full(ofs.shape, -1, dtype=np.int64)
    for k, s in enumerate(OUT_SIZES):
        branch[ofs == s] = k
    return branch


def _plan(branch):
    """Balanced routing plan: per-expert global index lists split evenly
    across cores, shared capacities, and segment layout [3,2,1,0]."""
    idx_all = {k: np.nonzero(branch == k)[0] for k in range(N_EXPERTS)}
    per_core = [int(-(-len(idx_all[k]) // N_CORES)) for k in range(N_EXPERTS)]
    caps = [int(-(-per_core[k] // 128) * 128) for k in range(N_EXPERTS)]
    # alignment: caps0 % 512, (caps1+2+3) % 512
    if caps[0] % 512:
        caps[0] += 512 - caps[0] % 512
    rem = (caps[1] + caps[2] + caps[3]) % 512
    if rem:
        for k in (1, 2, 3):  # pad the cheapest non-empty of e1..e3
            if caps[k]:
                caps[k] += 512 - rem
                break
        else:
            caps[0] += (512 - rem) if caps[0] else 0
    return idx_all, tuple(caps)


def kernel(x, weight, bias, out_feat_size):
    import ml_dtypes
    from concourse.bass_utils import run_bass_kernel_spmd

    bf16 = np.dtype(ml_dtypes.bfloat16)
    x = np.asarray(x, dtype=np.float32)
    weight = np.asarray(weight, dtype=np.float32)
    bias = np.asarray(bias, dtype=np.float32)
    B, T, D = x.shape
    assert D == IN_FEAT
    n_tok = B * T

    branch = _route(out_feat_size)
    idx_all, caps = _plan(branch)
    if sum(caps) == 0:
        return np.zeros((B, T, IN_FEAT), dtype=np.float32)

    # host-side weight/bias layout
    wt = np.empty((IN_FEAT, W_COLS), dtype=np.float32)
    bb = np.empty((W_COLS,), dtype=np.float32)
    for k, ok in enumerate(OUT_SIZES):
        wt[:, WOFF[k]:WOFF[k] + ok] = weight[k, :ok, :].T
        bb[WOFF[k]:WOFF[k] + ok] = bias[k, :ok]
    wt = wt.astype(bf16)
    bb128 = np.ascontiguousarray(np.broadcast_to(bb, (128, W_COLS)))

    x2 = x.reshape(n_tok, IN_FEAT).astype(bf16)
    tpad = sum(caps)
    seg_off = {}
    t0 = 0
    for k in (3, 2, 1, 0):
        if caps[k]:
            seg_off[k] = t0
            t0 += caps[k]

    in_maps = []
    core_slices = []  # per core: {expert: global idx array}
    for c in range(N_CORES):
        perm = np.zeros(tpad, dtype=np.int64)
        slices = {}
        for k, off in seg_off.items():
            idx = idx_all[k]
            m = int(-(-len(idx) // N_CORES))
            part = idx[c * m:(c + 1) * m]
            slices[k] = part
            if len(part):
                perm[off:off + len(part)] = part
                perm[off + len(part):off + caps[k]] = part[0]
        xtb = np.empty((tpad // CHUNK, IN_FEAT, CHUNK), dtype=bf16)
        for ci in range(tpad // CHUNK):
            np.copyto(xtb[ci], x2[perm[ci * CHUNK:(ci + 1) * CHUNK]].T)
        in_maps.append({"xt": xtb, "wt": wt, "bb": bb128})
        core_slices.append(slices)

    global _LAST_CAPS, _LAST_IN_MAPS
    _LAST_CAPS, _LAST_IN_MAPS = caps, in_maps

    nc = _get_nc(caps)
    res = run_bass_kernel_spmd(nc, in_maps, list(range(N_CORES))).results

    out = np.zeros((n_tok, IN_FEAT), dtype=np.float32)
    for c in range(N_CORES):
        for k, part in core_slices[c].items():
            n = len(part)
            if n == 0:
                continue
            ok = OUT_SIZES[k]
            if k == 0:
                out[part, :ok] = (res[c]["out0t"][:, :n].T.astype(np.float32)
                                  + bias[0, :ok])
            else:
                out[part, :ok] = res[c][f"out{k}"][:n].astype(np.float32)
            if ok < IN_FEAT:
                # reference semantics: bias tail beyond out_k (zero for the
                # shipped inputs, which pre-zero the bias)
                out[part, ok:] = bias[k, ok:]
    return out.reshape(B, T, IN_FEAT)


# revision 15
# speedup vs baseline: 1.2135x; 1.1620x over previous
"""MultiOutSizeLinear (MoE routed linear), Trainium2 x8 — weight-stationary.

Host side: route tokens to experts by ``out_feat_size``; balance each
expert's tokens evenly across the 8 cores (shared capacities so one SPMD
program serves all cores); gather + transpose each core's tokens into
chunk-blocked x^T in bf16. Device: the matmul stream is weight-stationary
with 4 interleaved PSUM accumulation chains:

  for each expert block (up to 4 chains of <=512 tokens):
    for each 128-col tile ct of the expert:
      for kk in 8 K-tiles:             # lhsT = W^T col tile, constant
        for chain j:                   #   across the 4-MM run
          psum_j[128 cols, n_j tok] += wT[ct,kk].T @ xT[kk, chain_j]

Consecutive MMs always hit different PSUM banks (4 chains x double buffer =
8 banks), the stationary operand only changes once per 4 MMs, and every MM
is N<=512 moving tokens. Measured on this part this stream shape sustains
the best PE clock under the chip's all-cores-active throttle.

Outputs are column-major per expert ([cts, 128 cols, caps] bf16); the host
transposes back. Bias is added on eviction from a host-prebroadcast
[128, 15*512] f32 tile.
"""

import sys
import numpy as np

sys.path.insert(0, "/opt/trn_rl_repo")

OUT_SIZES = (128, 256, 512, 1024)
N_EXPERTS = len(OUT_SIZES)
IN_FEAT = 1024
N_CORES = 8
K_TILES = IN_FEAT // 128
CHUNK = 512
WOFF = tuple(int(np.cumsum((0,) + OUT_SIZES)[k]) for k in range(N_EXPERTS))
W_COLS = sum(OUT_SIZES)
CBLOCKS = W_COLS // 128  # 15

_nc_cache: dict = {}


def _plan_chains(caps, seg_start, seg_order):
    """Per-expert chain list [(chunk, off, n, tokrow)] and block partition."""
    tpad = sum(caps)

    def expert_of(tok):
        for k in seg_order:
            if tok < seg_start[k] + caps[k]:
                return k
        raise AssertionError

    chains = {k: [] for k in seg_order}
    for c in range(tpad // CHUNK):
        g = 0
        while g < CHUNK:
            tok = c * CHUNK + g
            k = expert_of(tok)
            end = min(seg_start[k] + caps[k] - c * CHUNK, CHUNK)
            chains[k].append((c, g, end - g, tok - seg_start[k]))
            g = end

    def sizes(m):
        out = []
        while m > 0:
            if m == 5:
                out += [3, 2]
                m = 0
            elif m >= 4:
                out.append(4)
                m -= 4
            else:
                out.append(m)
                m = 0
        return out

    # flush[c] = list of (expert, [chains]) whose last chain is in chunk c
    flush = {}
    for k in seg_order:
        i = 0
        for s in sizes(len(chains[k])):
            grp = chains[k][i:i + s]
            i += s
            flush.setdefault(grp[-1][0], []).append((k, grp))
    return flush


def _build(caps, repeat=1, loop=None, xbufs=8, obufs=6):
    import concourse.bacc as bacc
    import concourse.mybir as mybir
    import concourse.tile as tile

    f32 = mybir.dt.float32
    bf16 = mybir.dt.bfloat16
    tpad = sum(caps)
    assert tpad % CHUNK == 0

    nc = bacc.Bacc(None, target_bir_lowering=False, debug=False)
    xt = nc.dram_tensor("xt", [tpad // CHUNK, IN_FEAT, CHUNK], bf16,
                        kind="ExternalInput")
    wt = nc.dram_tensor("wt", [IN_FEAT, W_COLS], bf16, kind="ExternalInput")
    bb = nc.dram_tensor("bb", [128, CBLOCKS * 512], f32, kind="ExternalInput")
    outs = {}
    for k in range(N_EXPERTS):
        if caps[k]:
            cts = OUT_SIZES[k] // 128
            outs[k] = nc.dram_tensor(f"out{k}c", [cts, 128, caps[k]], bf16,
                                     kind="ExternalOutput")

    seg_order = [k for k in (3, 2, 1, 0) if caps[k] > 0]
    seg_start = {}
    t0 = 0
    for k in seg_order:
        seg_start[k] = t0
        t0 += caps[k]

    flush = _plan_chains(caps, seg_start, seg_order)

    with tile.TileContext(nc) as tc:
        with (
            tc.tile_pool(name="const", bufs=1) as const,
            tc.tile_pool(name="xp", bufs=xbufs) as xp,
            tc.tile_pool(name="op", bufs=obufs) as op,
            tc.tile_pool(name="psw", bufs=2, space="PSUM") as psw,
        ):
            wt_sb = const.tile([128, K_TILES, W_COLS], bf16)
            nc.sync.dma_start(wt_sb[:], wt.rearrange("(kk p) n -> p kk n", p=128))
            bb_sb = const.tile([128, CBLOCKS * 512], f32)
            nc.sync.dma_start(bb_sb[:], bb[:])

            def emit_block(k, grp, x_tiles):
                cts = OUT_SIZES[k] // 128
                for ct in range(cts):
                    cb = (WOFF[k] + ct * 128) // 128
                    pss = [psw.tile([128, 512], f32, tag=f"w{j}",
                                    name=f"psw{j}") for j in range(len(grp))]
                    for kk in range(K_TILES):
                        for j, (ci, off, n, row) in enumerate(grp):
                            nc.tensor.matmul(
                                pss[j][:, :n],
                                wt_sb[:, kk,
                                      WOFF[k] + ct * 128:WOFF[k] + ct * 128 + 128],
                                x_tiles[ci][:, kk, off:off + n],
                                start=(kk == 0), stop=(kk == K_TILES - 1))
                    for j, (ci, off, n, row) in enumerate(grp):
                        o_sb = op.tile([128, 512], bf16, tag="o", name="o_sb")
                        nc.vector.tensor_add(
                            o_sb[:, :n], pss[j][:, :n],
                            bb_sb[:, cb * 512:cb * 512 + n])
                        nc.sync.dma_start(outs[k][ct, :, row:row + n],
                                          o_sb[:, :n])

            def body():
                x_tiles = {}
                for c in range(tpad // CHUNK):
                    x_sb = xp.tile([128, K_TILES, CHUNK], bf16, tag="x",
                                   name="x_sb")
                    nc.scalar.dma_start(
                        x_sb[:],
                        xt[c].rearrange("(kk p) t -> p kk t", p=128))
                    x_tiles[c] = x_sb
                    for k, grp in flush.get(c, []):
                        emit_block(k, grp, x_tiles)

            if loop:
                with tc.For_i(0, loop, 1):
                    for _ in range(repeat):
                        body()
            else:
                for _ in range(repeat):
                    body()
    nc.compile()
    return nc


def _get_nc(caps, repeat=1, loop=None):
    key = (tuple(caps), repeat, loop)
    if key not in _nc_cache:
        _nc_cache[key] = _build(caps, repeat=repeat, loop=loop)
    return _nc_cache[key]


def _route(out_feat_size):
    ofs = np.asarray(out_feat_size).astype(np.int64).reshape(-1)
    branch = np.full(ofs.shape, -1, dtype=np.int64)
    for k, s in enumerate(OUT_SIZES):
        branch[ofs == s] = k
    return branch


def _plan(branch):
    idx_all = {k: np.nonzero(branch == k)[0] for k in range(N_EXPERTS)}
    per_core = [int(-(-len(idx_all[k]) // N_CORES)) for k in range(N_EXPERTS)]
    caps = [int(-(-per_core[k] // 128) * 128) for k in range(N_EXPERTS)]
    rem = sum(caps) % 512
    if rem:
        for k in (0, 1, 2, 3):  # pad the cheapest non-empty expert
            if caps[k]:
                caps[k] += 512 - rem
                break
    return idx_all, tuple(caps)


def kernel(x, weight, bias, out_feat_size):
    import ml_dtypes
    from concourse.bass_utils import run_bass_kernel_spmd

    bf16 = np.dtype(ml_dtypes.bfloat16)
    x = np.asarray(x, dtype=np.float32)
    weight = np.asarray(weight, dtype=np.float32)
    bias = np.asarray(bias, dtype=np.float32)
    B, T, D = x.shape
    assert D == IN_FEAT
    n_tok = B * T

    branch = _route(out_feat_size)
    idx_all, caps = _plan(branch)
    if sum(caps) == 0:
        return np.zeros((B, T, IN_FEAT), dtype=np.float32)

    wt = np.empty((IN_FEAT, W_COLS), dtype=np.float32)
    bb = np.empty((W_COLS,), dtype=np.float32)
    for k, ok in enumerate(OUT_SIZES):
        wt[:, WOFF[k]:WOFF[k] + ok] = weight[k, :ok, :].T
        bb[WOFF[k]:WOFF[k] + ok] = bias[k, :ok]
    wt = wt.astype(bf16)
    # bias pre-broadcast: [128, cb*512 + t] = bb[cb*128 + p]
    bb_bc = np.ascontiguousarray(
        np.repeat(bb.reshape(CBLOCKS, 128).T[:, :, None], 512, axis=2)
        .reshape(128, CBLOCKS * 512))

    x2 = x.reshape(n_tok, IN_FEAT).astype(bf16)
    tpad = sum(caps)
    seg_off = {}
    t0 = 0
    for k in (3, 2, 1, 0):
        if caps[k]:
            seg_off[k] = t0
            t0 += caps[k]

    in_maps = []
    core_slices = []
    for c in range(N_CORES):
        perm = np.zeros(tpad, dtype=np.int64)
        slices = {}
        for k, off in seg_off.items():
            idx = idx_all[k]
            m = int(-(-len(idx) // N_CORES))
            part = idx[c * m:(c + 1) * m]
            slices[k] = part
            if len(part):
                perm[off:off + len(part)] = part
                perm[off + len(part):off + caps[k]] = part[0]
        xtb = np.empty((tpad // CHUNK, IN_FEAT, CHUNK), dtype=bf16)
        for ci in range(tpad // CHUNK):
            np.copyto(xtb[ci], x2[perm[ci * CHUNK:(ci + 1) * CHUNK]].T)
        in_maps.append({"xt": xtb, "wt": wt, "bb": bb_bc})
        core_slices.append(slices)

    global _LAST_CAPS, _LAST_IN_MAPS
    _LAST_CAPS, _LAST_IN_MAPS = caps, in_maps

    nc = _get_nc(caps)
    res = run_bass_kernel_spmd(nc, in_maps, list(range(N_CORES))).results

    out = np.zeros((n_tok, IN_FEAT), dtype=np.float32)
    for c in range(N_CORES):
        for k, part in core_slices[c].items():
            n = len(part)
            if n == 0:
                continue
            ok = OUT_SIZES[k]
            r = res[c][f"out{k}c"][:, :, :n].astype(np.float32)
            out[part, :ok] = np.transpose(r, (2, 0, 1)).reshape(n, ok)
            if ok < IN_FEAT:
                out[part, ok:] = bias[k, ok:]
    return out.reshape(B, T, IN_FEAT)


# revision 20
# speedup vs baseline: 1.2881x; 1.0615x over previous
"""MultiOutSizeLinear (MoE routed linear), Trainium2 x8 — weight-stationary.

Host side: route tokens to experts by ``out_feat_size``; balance each
expert's tokens evenly across the 8 cores (shared capacities so one SPMD
program serves all cores); gather + transpose each core's tokens into
chunk-blocked x^T in bf16. Device: the matmul stream is weight-stationary
with 4 interleaved PSUM accumulation chains:

  for each expert block (up to 4 chains of <=512 tokens):
    for each 128-col tile ct of the expert:
      for kk in 8 K-tiles:             # lhsT = W^T col tile, constant
        for chain j:                   #   across the 4-MM run
          psum_j[128 cols, n_j tok] += wT[ct,kk].T @ xT[kk, chain_j]

Consecutive MMs always hit different PSUM banks (4 chains x double buffer =
8 banks), the stationary operand only changes once per 4 MMs, and every MM
is N<=512 moving tokens. Measured on this part this stream shape sustains
the best PE clock under the chip's all-cores-active throttle.

Outputs are column-major per expert ([cts, 128 cols, caps] bf16); the host
transposes back. Bias is added on eviction from a host-prebroadcast
[128, 15*512] f32 tile.
"""

import sys
import numpy as np

sys.path.insert(0, "/opt/trn_rl_repo")

OUT_SIZES = (128, 256, 512, 1024)
N_EXPERTS = len(OUT_SIZES)
IN_FEAT = 1024
N_CORES = 8
K_TILES = IN_FEAT // 128
CHUNK = 512
WOFF = tuple(int(np.cumsum((0,) + OUT_SIZES)[k]) for k in range(N_EXPERTS))
W_COLS = sum(OUT_SIZES)
CBLOCKS = W_COLS // 128  # 15

_nc_cache: dict = {}


def _plan_chains(caps, seg_start, seg_order):
    """Per-expert chain list [(chunk, off, n, tokrow)] and block partition."""
    tpad = sum(caps)

    def expert_of(tok):
        for k in seg_order:
            if tok < seg_start[k] + caps[k]:
                return k
        raise AssertionError

    chains = {k: [] for k in seg_order}
    for c in range(tpad // CHUNK):
        g = 0
        while g < CHUNK:
            tok = c * CHUNK + g
            k = expert_of(tok)
            end = min(seg_start[k] + caps[k] - c * CHUNK, CHUNK)
            chains[k].append((c, g, end - g, tok - seg_start[k]))
            g = end

    def sizes(m, lead2=False):
        # lead2: start with a 2-chain block so the first MMs begin after
        # only two chunk DMAs (faster pipeline fill at the loop boundary)
        out = []
        if lead2 and m >= 5:
            out.append(2)
            m -= 2
        while m > 0:
            if m == 5:
                out += [3, 2]
                m = 0
            elif m >= 4:
                out.append(4)
                m -= 4
            else:
                out.append(m)
                m = 0
        return out

    # flush[c] = list of (expert, [chains]) whose last chain is in chunk c
    flush = {}
    for k in seg_order:
        i = 0
        for s in sizes(len(chains[k]), lead2=(k == seg_order[0])):
            grp = chains[k][i:i + s]
            i += s
            flush.setdefault(grp[-1][0], []).append((k, grp))
    return flush


def _build(caps, repeat=1, loop=None, xbufs=10, obufs=6,
           staggered=True):
    import concourse.bacc as bacc
    import concourse.mybir as mybir
    import concourse.tile as tile

    f32 = mybir.dt.float32
    bf16 = mybir.dt.bfloat16
    tpad = sum(caps)
    assert tpad % CHUNK == 0

    nc = bacc.Bacc(None, target_bir_lowering=False, debug=False)
    xt = nc.dram_tensor("xt", [tpad // CHUNK, IN_FEAT, CHUNK], bf16,
                        kind="ExternalInput")
    wt = nc.dram_tensor("wt", [IN_FEAT, W_COLS], bf16, kind="ExternalInput")
    bb = nc.dram_tensor("bb", [128, CBLOCKS * 512], f32, kind="ExternalInput")
    outs = {}
    for k in range(N_EXPERTS):
        if caps[k]:
            cts = OUT_SIZES[k] // 128
            outs[k] = nc.dram_tensor(f"out{k}c", [cts, 128, caps[k]], bf16,
                                     kind="ExternalOutput")

    seg_order = [k for k in (3, 2, 1, 0) if caps[k] > 0]
    seg_start = {}
    t0 = 0
    for k in seg_order:
        seg_start[k] = t0
        t0 += caps[k]

    flush = _plan_chains(caps, seg_start, seg_order)

    with tile.TileContext(nc) as tc:
        with (
            tc.tile_pool(name="const", bufs=1) as const,
            tc.tile_pool(name="xp", bufs=xbufs) as xp,
            tc.tile_pool(name="op", bufs=obufs) as op,
            tc.tile_pool(name="psw", bufs=2, space="PSUM") as psw,
        ):
            wt_sb = const.tile([128, K_TILES, W_COLS], bf16)
            nc.sync.dma_start(wt_sb[:], wt.rearrange("(kk p) n -> p kk n", p=128))
            bb_sb = const.tile([128, CBLOCKS * 512], f32)
            nc.sync.dma_start(bb_sb[:], bb[:])

            def emit_block(k, grp, x_tiles):
                cts = OUT_SIZES[k] // 128
                for ct in range(cts):
                    cb = (WOFF[k] + ct * 128) // 128
                    pss = [psw.tile([128, 512], f32, tag=f"w{j}",
                                    name=f"psw{j}") for j in range(len(grp))]
                    for kk in range(K_TILES):
                        for j, (ci, off, n, row) in enumerate(grp):
                            nc.tensor.matmul(
                                pss[j][:, :n],
                                wt_sb[:, kk,
                                      WOFF[k] + ct * 128:WOFF[k] + ct * 128 + 128],
                                x_tiles[ci][:, kk, off:off + n],
                                start=(kk == 0), stop=(kk == K_TILES - 1))
                    for j, (ci, off, n, row) in enumerate(grp):
                        o_sb = op.tile([128, 512], bf16, tag="o", name="o_sb")
                        nc.vector.tensor_add(
                            o_sb[:, :n], pss[j][:, :n],
                            bb_sb[:, cb * 512:cb * 512 + n])
                        nc.sync.dma_start(outs[k][ct, :, row:row + n],
                                          o_sb[:, :n])

            def body():
                x_tiles = {}
                for c in range(tpad // CHUNK):
                    x_sb = xp.tile([128, K_TILES, CHUNK], bf16, tag="x",
                                   name="x_sb")
                    nc.scalar.dma_start(
                        x_sb[:],
                        xt[c].rearrange("(kk p) t -> p kk t", p=128))
                    x_tiles[c] = x_sb
                    for k, grp in flush.get(c, []):
                        emit_block(k, grp, x_tiles)

            if loop:
                with tc.For_i(0, loop, 1, staggered_reset=staggered):
                    for _ in range(repeat):
                        body()
            else:
                for _ in range(repeat):
                    body()
    nc.compile()
    return nc


def _get_nc(caps, repeat=1, loop=None):
    key = (tuple(caps), repeat, loop)
    if key not in _nc_cache:
        _nc_cache[key] = _build(caps, repeat=repeat, loop=loop)
    return _nc_cache[key]


def _route(out_feat_size):
    ofs = np.asarray(out_feat_size).astype(np.int64).reshape(-1)
    branch = np.full(ofs.shape, -1, dtype=np.int64)
    for k, s in enumerate(OUT_SIZES):
        branch[ofs == s] = k
    return branch


def _plan(branch):
    idx_all = {k: np.nonzero(branch == k)[0] for k in range(N_EXPERTS)}
    per_core = [int(-(-len(idx_all[k]) // N_CORES)) for k in range(N_EXPERTS)]
    caps = [int(-(-per_core[k] // 128) * 128) for k in range(N_EXPERTS)]
    rem = sum(caps) % 512
    if rem:
        for k in (0, 1, 2, 3):  # pad the cheapest non-empty expert
            if caps[k]:
                caps[k] += 512 - rem
                break
    return idx_all, tuple(caps)


def kernel(x, weight, bias, out_feat_size):
    import ml_dtypes
    from concourse.bass_utils import run_bass_kernel_spmd

    bf16 = np.dtype(ml_dtypes.bfloat16)
    x = np.asarray(x, dtype=np.float32)
    weight = np.asarray(weight, dtype=np.float32)
    bias = np.asarray(bias, dtype=np.float32)
    B, T, D = x.shape
    assert D == IN_FEAT
    n_tok = B * T

    branch = _route(out_feat_size)
    idx_all, caps = _plan(branch)
    if sum(caps) == 0:
        return np.zeros((B, T, IN_FEAT), dtype=np.float32)

    wt = np.empty((IN_FEAT, W_COLS), dtype=np.float32)
    bb = np.empty((W_COLS,), dtype=np.float32)
    for k, ok in enumerate(OUT_SIZES):
        wt[:, WOFF[k]:WOFF[k] + ok] = weight[k, :ok, :].T
        bb[WOFF[k]:WOFF[k] + ok] = bias[k, :ok]
    wt = wt.astype(bf16)
    # bias pre-broadcast: [128, cb*512 + t] = bb[cb*128 + p]
    bb_bc = np.ascontiguousarray(
        np.repeat(bb.reshape(CBLOCKS, 128).T[:, :, None], 512, axis=2)
        .reshape(128, CBLOCKS * 512))

    x2 = x.reshape(n_tok, IN_FEAT).astype(bf16)
    tpad = sum(caps)
    seg_off = {}
    t0 = 0
    for k in (3, 2, 1, 0):
        if caps[k]:
            seg_off[k] = t0
            t0 += caps[k]

    in_maps = []
    core_slices = []
    for c in range(N_CORES):
        perm = np.zeros(tpad, dtype=np.int64)
        slices = {}
        for k, off in seg_off.items():
            idx = idx_all[k]
            m = int(-(-len(idx) // N_CORES))
            part = idx[c * m:(c + 1) * m]
            slices[k] = part
            if len(part):
                perm[off:off + len(part)] = part
                perm[off + len(part):off + caps[k]] = part[0]
        xtb = np.empty((tpad // CHUNK, IN_FEAT, CHUNK), dtype=bf16)
        for ci in range(tpad // CHUNK):
            np.copyto(xtb[ci], x2[perm[ci * CHUNK:(ci + 1) * CHUNK]].T)
        in_maps.append({"xt": xtb, "wt": wt, "bb": bb_bc})
        core_slices.append(slices)

    global _LAST_CAPS, _LAST_IN_MAPS
    _LAST_CAPS, _LAST_IN_MAPS = caps, in_maps

    nc = _get_nc(caps)
    res = run_bass_kernel_spmd(nc, in_maps, list(range(N_CORES))).results

    out = np.zeros((n_tok, IN_FEAT), dtype=np.float32)
    for c in range(N_CORES):
        for k, part in core_slices[c].items():
            n = len(part)
            if n == 0:
                continue
            ok = OUT_SIZES[k]
            r = res[c][f"out{k}c"][:, :, :n].astype(np.float32)
            out[part, :ok] = np.transpose(r, (2, 0, 1)).reshape(n, ok)
            if ok < IN_FEAT:
                out[part, ok:] = bias[k, ok:]
    return out.reshape(B, T, IN_FEAT)
